# revision 22
# baseline (speedup 1.0000x reference)
# Trainium2 Bass kernel for nn_AttnBlock (GroupNorm + single-head NxN attention + proj + residual).
#
# Sharding: 8 cores = batch (4) x token-half (2). Each core receives its batch's
# x as (C=256, N=4096) with the token axis rolled so that the core's 2048 query
# tokens sit at local positions 0..2047. GroupNorm stats / k / v are
# token-permutation invariant, so every core computes GN and full k/v locally
# (≈19% redundant FLOPs) and attention rows only for its half — no collectives.
#
# Structure notes:
#   - GroupNorm (128 groups of 2 adjacent channels) is computed as per-channel
#     bn_stats, pair-combined + broadcast with one tiny matmul against a
#     0.5-scaled pairing matrix. The affine normalize hn = a*x + b is FOLDED
#     into the q/k/v weights (w' = a_c * w, bias' += w @ b), so projections
#     consume raw x straight off the DMA — no normalize pass on the critical
#     path.
#   - scores^T computed pre-transposed (key tokens on partitions); exp via ACT
#     (scale=C^-0.5 fused, no max-subtraction: scores ~ N(0,1)) into bf16
#     tiles; ones-column on V makes the softmax denominator fall out of the
#     attention matmul; h2 accumulation runs kt-outer over 4 live PSUM banks so
#     PE consumption paces ACT production.
#   - The layout-scrambling reshape ((B,N,C)->(B,H,W,C)->(B,C,H,W)->(B,HW,C))
#     plus final (B,HW,C)->(B,C,H,W) resolves per 256-token block t to:
#       out[c, t*256+u] = sum_cc h2[t*256+cc, c] * wo[u, cc] + bo[u] + x[c, t*256+u]
#     i.e. a matmul with the h2 block itself as the stationary operand; it runs
#     per chunk so the kernel tail stays busy.
#   - fp32r everywhere matmul-facing (full PE rate at free-dim >= 256); the BIR
#     verifier requires fp32r-matmul operands to be *produced* as fp32r, so
#     those DRAM tensors / SBUF tiles are typed float32r natively.

import numpy as np

B, C, HH, WW = 4, 256, 64, 64
N = HH * WW           # 4096 tokens
NL = N // 2           # 2048 local query tokens per core
P = 128
EPS = 1e-5
NCORES = 8

_CACHE = {}

# fp8e4m3 + DoubleRow for the scores matmul: fuses the 2-tile channel
# contraction into one matmul at 2x PE rate. k/q ~ N(0,1) sit far inside
# fp8e4m3 range; overall output error stays ~1e-3 (residual-dominated).
SCORES_FP8 = True
# fp8 + DoubleRow for the attention-value matmul too: exp tiles and V stored
# fp8 in paired k-tile layout; V rows padded to 272 so the DoubleRow ko-stride
# is 16B-aligned. Softmax numerator/denominator stay consistent (both use the
# quantized weights), so this perturbs attention weights by ~fp8 eps only.
H2_FP8 = True


def _build_nc(reps=1):
    import concourse.bass as bass
    import concourse.tile as tile
    from concourse import bacc, mybir

    f32 = mybir.dt.float32
    f32r = mybir.dt.float32r
    bf16 = mybir.dt.bfloat16
    fp8 = mybir.dt.float8e4
    Alu = mybir.AluOpType
    Act = mybir.ActivationFunctionType

    nc = bacc.Bacc("TRN2", target_bir_lowering=False, debug=False, num_devices=NCORES)

    x_d = nc.dram_tensor("x", [C, N], f32r, kind="ExternalInput")
    wqt_d = nc.dram_tensor("wqt", [C, C], f32r, kind="ExternalInput")
    wkt_d = nc.dram_tensor("wkt", [C, C], f32r, kind="ExternalInput")
    wvt_d = nc.dram_tensor("wvt", [C, C], f32r, kind="ExternalInput")
    wot_d = nc.dram_tensor("wot", [C, C], f32r, kind="ExternalInput")
    bq_d = nc.dram_tensor("bq", [C], f32, kind="ExternalInput")
    bk_d = nc.dram_tensor("bk", [C], f32, kind="ExternalInput")
    bv_d = nc.dram_tensor("bv", [C], f32, kind="ExternalInput")
    bo_d = nc.dram_tensor("bo", [C], f32, kind="ExternalInput")
    gnw_d = nc.dram_tensor("gnw", [C], f32, kind="ExternalInput")
    gnb_d = nc.dram_tensor("gnb", [C], f32, kind="ExternalInput")
    pairm_d = nc.dram_tensor("pairm", [P, P], f32, kind="ExternalInput")  # 0.5-scaled
    out_d = nc.dram_tensor("out", [C, NL], f32, kind="ExternalOutput")

    xa = x_d.ap()
    outa = out_d.ap()

    KT = N // P    # 32 key-token tiles
    SCH = 512      # free-dim chunk for projections / scores
    NCH = N // SCH     # 8
    QCH = NL // SCH    # 4

    with tile.TileContext(nc) as tc:
        from contextlib import ExitStack

        with ExitStack() as ctx:
            consts = ctx.enter_context(tc.tile_pool(name="consts", bufs=1))
            big = ctx.enter_context(tc.tile_pool(name="big", bufs=1))
            small = ctx.enter_context(tc.tile_pool(name="small", bufs=1))
            etp = ctx.enter_context(tc.tile_pool(name="etp", bufs=36))
            outp = ctx.enter_context(tc.tile_pool(name="outp", bufs=4))
            psum = ctx.enter_context(tc.tile_pool(name="psum", bufs=1, space="PSUM"))

            loop_cm = tc.For_i(0, reps, 1) if reps > 1 else None
            if loop_cm is not None:
                ctx.enter_context(loop_cm)

            # ---------- x DMA (ci-interleaved 512-token chunks) + bn_stats trailing ----------
            xh = [big.tile([P, N], f32r, name=f"xh{ci}") for ci in range(2)]
            st6 = [small.tile([P, NCH, 6], f32, name=f"st6_{ci}") for ci in range(2)]
            for s in range(NCH):
                sl = slice(s * SCH, (s + 1) * SCH)
                for ci in range(2):
                    nc.sync.dma_start(xh[ci][:, sl], xa[ci * P:(ci + 1) * P, sl])
                for ci in range(2):
                    nc.vector.bn_stats(out=st6[ci][:, s, :], in_=xh[ci][:, sl])

            # ---------- constants ----------
            pairm_sb = consts.tile([P, P], f32, name="pairm_sb")
            nc.sync.dma_start(pairm_sb[:], pairm_d.ap())

            w_sb = {}
            for wname, wd in (("k", wkt_d), ("q", wqt_d), ("v", wvt_d), ("o", wot_d)):
                for ci in range(2):
                    t = consts.tile([P, C], f32r, name=f"w{wname}t_sb{ci}")
                    nc.sync.dma_start(t[:], wd.ap()[ci * P:(ci + 1) * P, :])
                    w_sb[wname, ci] = t

            def chan_tiles(d, nm):
                ts = []
                for ci in range(2):
                    t = consts.tile([P, 1], f32, name=f"{nm}_sb{ci}")
                    nc.sync.dma_start(t[:], d.ap()[ci * P:(ci + 1) * P].unsqueeze(-1))
                    ts.append(t)
                return ts

            bq_sb = chan_tiles(bq_d, "bq")
            bk_sb = chan_tiles(bk_d, "bk")
            gnw_sb = chan_tiles(gnw_d, "gnw")
            gnb_sb = chan_tiles(gnb_d, "gnb")

            bv_sb = consts.tile([P, C], f32, name="bv_sb")
            nc.sync.dma_start(
                bv_sb[:], bass.AP(tensor=bv_d, offset=0, ap=[[0, P], [1, C]])
            )
            bo_sb = consts.tile([P, C], f32, name="bo_sb")
            nc.sync.dma_start(
                bo_sb[:], bass.AP(tensor=bo_d, offset=0, ap=[[0, P], [1, C]])
            )

            eps_sb = consts.tile([P, 1], f32, name="eps_sb")
            nc.vector.memset(eps_sb[:], EPS)
            shift_sb = consts.tile([P, 1], f32, name="shift_sb")
            nc.vector.memset(shift_sb[:], -4.0)

            # ---------- GroupNorm coefficients a, b (no normalize pass) ----------
            ab = []
            for ci in range(2):
                mv = small.tile([P, 2], f32, name=f"mv_{ci}")
                nc.vector.bn_aggr(out=mv[:], in_=st6[ci][:])
                stats2 = small.tile([P, 2], f32, name=f"stats2_{ci}")
                nc.vector.tensor_mul(stats2[:, 1:2], mv[:, 0:1], mv[:, 0:1])
                nc.vector.tensor_add(stats2[:, 1:2], stats2[:, 1:2], mv[:, 1:2])
                nc.vector.tensor_copy(stats2[:, 0:1], mv[:, 0:1])
                # pairm is 0.5-scaled -> [mean_g, E_g[x^2]] broadcast to both
                # partitions of each channel pair in one matmul
                pair_ps = psum.tile([P, 2], f32, name=f"pair_ps{ci}", tag="pps", bufs=2)
                nc.tensor.matmul(pair_ps[:], pairm_sb[:], stats2[:], start=True, stop=True)
                pairs = small.tile([P, 2], f32, name=f"pairs{ci}")
                nc.vector.tensor_copy(pairs[:], pair_ps[:])
                var_g = small.tile([P, 1], f32, name=f"var_g{ci}")
                nc.vector.tensor_mul(var_g[:], pairs[:, 0:1], pairs[:, 0:1])
                nc.vector.tensor_tensor(var_g[:], pairs[:, 1:2], var_g[:], Alu.subtract)
                sqv = small.tile([P, 1], f32, name=f"sqv{ci}")
                nc.scalar.activation(sqv[:], var_g[:], Act.Sqrt, bias=eps_sb[:], scale=1.0)
                rstd = small.tile([P, 1], f32, name=f"rstd{ci}")
                nc.vector.reciprocal(rstd[:], sqv[:])
                # a = rstd*gn_w ; b = gn_b - mean*a
                a_t = small.tile([P, 1], f32, name=f"a_t{ci}")
                nc.vector.tensor_mul(a_t[:], rstd[:], gnw_sb[ci][:])
                b_t = small.tile([P, 1], f32r, name=f"b_t{ci}")
                nc.vector.tensor_mul(b_t[:], pairs[:, 0:1], a_t[:])
                nc.vector.tensor_tensor(b_t[:], gnb_sb[ci][:], b_t[:], Alu.subtract)
                ab.append((a_t, b_t))

            # ---------- fold GN affine into q/k/v weights ----------
            # w'[c, co] = a_c * w^T[c, co]; bias'[co] = base[co] + sum_c b_c w^T[c, co]
            ws = {}
            for wname in ("k", "q", "v"):
                for ci in range(2):
                    t = consts.tile([P, C], f32r, name=f"w{wname}s_sb{ci}")
                    nc.vector.tensor_scalar_mul(t[:], w_sb[wname, ci][:], ab[ci][0][:])
                    ws[wname, ci] = t

            # b padded to 2 columns (fp32r matmul needs even/non-unit free dim)
            b_pad = []
            for ci in range(2):
                bp = small.tile([P, 2], f32r, name=f"b_pad{ci}")
                nc.vector.tensor_copy(bp[:], ab[ci][1][:].to_broadcast((P, 2)))
                b_pad.append(bp)

            kq_bias = {}
            for wname, base_sb in (("k", bk_sb), ("q", bq_sb)):
                for co in range(2):
                    bps = psum.tile([P, 2], f32, name=f"bps_{wname}{co}", tag="pps", bufs=2)
                    nc.tensor.matmul(bps[:], w_sb[wname, 0][:, co * P:(co + 1) * P],
                                     b_pad[0][:], start=True, stop=False)
                    nc.tensor.matmul(bps[:], w_sb[wname, 1][:, co * P:(co + 1) * P],
                                     b_pad[1][:], start=False, stop=True)
                    bt = small.tile([P, 1], f32, name=f"bias_{wname}{co}")
                    nc.vector.tensor_tensor(bt[:], bps[:, 0:1], base_sb[co][:], Alu.add)
                    kq_bias[wname, co] = bt

            # v bias row, replicated across partitions for free: use lhsT =
            # b broadcast along M (so every output partition gets the same row)
            # vbias[m, u] = sum_c b_c * wv^T[c, u] + bv[u]
            b_rep = []
            for ci in range(2):
                br = small.tile([P, P], f32r, name=f"b_rep{ci}")
                nc.vector.tensor_copy(br[:], ab[ci][1][:].to_broadcast((P, P)))
                b_rep.append(br)
            vbps = psum.tile([P, C], f32, name="vbps", tag="pps", bufs=2)
            nc.tensor.matmul(vbps[:], b_rep[0][:], w_sb["v", 0][:], start=True, stop=False)
            nc.tensor.matmul(vbps[:], b_rep[1][:], w_sb["v", 1][:], start=False, stop=True)
            vbias_sb = consts.tile([P, C], f32, name="vbias_sb")
            nc.vector.tensor_tensor(vbias_sb[:], vbps[:], bv_sb[:], Alu.add)

            # ---------- k/q projections (fp8 pair layout for DoubleRow scores) ----------
            if SCORES_FP8:
                kT_pair = big.tile([P, 2, N], fp8, name="kT_pair")
                qT_pair = big.tile([P, 2, NL], fp8, name="qT_pair")
                kT = [kT_pair[:, ci, :] for ci in range(2)]
                qT = [qT_pair[:, ci, :] for ci in range(2)]
            else:
                kT = [big.tile([P, N], f32r, name=f"kT{ci}") for ci in range(2)]
                qT = [big.tile([P, NL], f32r, name=f"qT{ci}") for ci in range(2)]

            def emit_kproj(s):
                sl = slice(s * SCH, (s + 1) * SCH)
                for co in range(2):
                    ps = psum.tile([P, SCH], f32, name=f"kps_{co}_{s}", tag="sps", bufs=2)
                    nc.tensor.matmul(ps[:], ws["k", 0][:, co * P:(co + 1) * P],
                                     xh[0][:, sl], start=True, stop=False)
                    nc.tensor.matmul(ps[:], ws["k", 1][:, co * P:(co + 1) * P],
                                     xh[1][:, sl], start=False, stop=True)
                    nc.vector.tensor_scalar_add(kT[co][:, sl], ps[:], kq_bias["k", co][:])

            def emit_qproj(s):
                sl = slice(s * SCH, (s + 1) * SCH)
                for co in range(2):
                    ps = psum.tile([P, SCH], f32, name=f"qps_{co}_{s}", tag="sps", bufs=2)
                    nc.tensor.matmul(ps[:], ws["q", 0][:, co * P:(co + 1) * P],
                                     xh[0][:, sl], start=True, stop=False)
                    nc.tensor.matmul(ps[:], ws["q", 1][:, co * P:(co + 1) * P],
                                     xh[1][:, sl], start=False, stop=True)
                    nc.vector.tensor_scalar_add(qT[co][:, sl], ps[:], kq_bias["q", co][:])

            # v in (token on partitions, channel free) layout with ones column
            if H2_FP8:
                CP = 272  # C+1 padded to a 16B multiple for the DoubleRow ko-stride
                v_sb = big.tile([P, KT // 2, 2, CP], fp8, name="v_sb")
                nc.vector.memset(v_sb[:, :, :, C:], 0.0)
                nc.vector.memset(v_sb[:, :, :, C:C + 1], 1.0)
            else:
                v_sb = big.tile([P, KT, C + 1], bf16, name="v_sb")
                nc.vector.memset(v_sb[:, :, C:C + 1], 1.0)

            def emit_v(kt):
                tsl = slice(kt * P, (kt + 1) * P)
                ps = psum.tile([P, C], f32, name=f"vps_{kt}", tag="pps", bufs=2)
                nc.tensor.matmul(ps[:], xh[0][:, tsl], ws["v", 0][:],
                                 start=True, stop=False)
                nc.tensor.matmul(ps[:], xh[1][:, tsl], ws["v", 1][:],
                                 start=False, stop=True)
                if H2_FP8:
                    nc.vector.tensor_tensor(v_sb[:, kt // 2, kt % 2, 0:C], ps[:],
                                            vbias_sb[:], Alu.add)
                else:
                    nc.vector.tensor_tensor(v_sb[:, kt, 0:C], ps[:], vbias_sb[:], Alu.add)

            # ---------- attention, software-pipelined ----------
            # Chunk qc's h2 (PE-heavy) is emitted interleaved with chunk qc+1's
            # scores+exp (ACT-heavy) so neither engine idles; the k/q/v
            # projections fill the PE during chunk 0's ACT-paced scores.
            scale = float(C) ** -0.5
            et_chunks = [[None] * (KT // 2) for _ in range(QCH)]

            def emit_score_pair(qc, ktp):
                # two DoubleRow score matmuls into one 2-bank PSUM slot, then a
                # single batched exp writes the whole fp8 et pair tile
                qsl = slice(qc * SCH, (qc + 1) * SCH)
                ps2 = psum.tile([P, 2, SCH], f32, name=f"sps_{qc}_{ktp}", tag="sps", bufs=2)
                for j in range(2):
                    kt = 2 * ktp + j
                    nc.tensor.matmul(ps2[:, j, :], kT_pair[:, :, kt * P:(kt + 1) * P],
                                     qT_pair[:, :, qsl], start=True, stop=True,
                                     perf_mode=mybir.MatmulPerfMode.DoubleRow)
                ets = et_chunks[qc]
                ets[ktp] = etp.tile([P, 2, SCH], fp8, name=f"et_{qc}_{ktp}", tag="et")
                nc.scalar.activation(ets[ktp][:], ps2[:], Act.Exp,
                                     scale=scale, bias=shift_sb[:])

            # prologue: q chunk 0, then per 512-token slab: k-proj followed by
            # its 4 score blocks for chunk 0, v tiles, remaining q chunks
            emit_qproj(0)
            for s in range(NCH):
                emit_kproj(s)
                if s >= 5:
                    emit_qproj(s - 4)
                for ktp in range(2 * s, 2 * s + 2):
                    emit_score_pair(0, ktp)
                    emit_v(2 * ktp)
                    emit_v(2 * ktp + 1)

            def emit_final(rr):
                for mt in range(2):
                    msl = slice(mt * P, (mt + 1) * P)
                    usl = slice(rr * C, (rr + 1) * C)
                    ps = psum.tile([P, C], f32, name=f"ops_{rr}_{mt}", tag="pps", bufs=2)
                    nc.tensor.matmul(ps[:], h2[2 * rr][:, msl], w_sb["o", 0][:],
                                     start=True, stop=False)
                    nc.tensor.matmul(ps[:], h2[2 * rr + 1][:, msl], w_sb["o", 1][:],
                                     start=False, stop=True)
                    osb = outp.tile([P, C], f32, name=f"osb_{rr}_{mt}", tag="osb", bufs=4)
                    nc.vector.tensor_tensor(osb[:], ps[:], bo_sb[:], Alu.add)
                    nc.vector.tensor_tensor(osb[:], osb[:], xh[mt][:, usl], Alu.add)
                    nc.sync.dma_start(outa[mt * P:(mt + 1) * P, usl], osb[:])

            h2 = []
            for qc in range(QCH):
                ets = et_chunks[qc]
                for half in range(2):
                    hpss = [
                        psum.tile([P, CP], f32, name=f"hps_{qc}_{half}_{j}",
                                  tag="hps", bufs=2)
                        for j in range(2)
                    ]
                    for ktp in range(KT // 2):
                        for j in range(2):
                            qt = 2 * half + j
                            nc.tensor.matmul(hpss[j][:],
                                             ets[ktp][:, :, qt * P:(qt + 1) * P],
                                             v_sb[:, ktp, :, :],
                                             start=(ktp == 0), stop=(ktp == KT // 2 - 1),
                                             perf_mode=mybir.MatmulPerfMode.DoubleRow)
                        if qc + 1 < QCH and ktp % 2 == half:
                            emit_score_pair(qc + 1, ktp)
                    for j in range(2):
                        qt = 2 * half + j
                        rec = small.tile([P, 1], f32, name=f"rec_{qc}_{qt}", tag="rec", bufs=4)
                        nc.vector.reciprocal(rec[:], hpss[j][:, C:C + 1])
                        h2t = big.tile([P, C], f32r, name=f"h2_{qc}_{qt}", tag="h2", bufs=6)
                        nc.vector.tensor_scalar_mul(h2t[:], hpss[j][:, 0:C], rec[:])
                        h2.append(h2t)
                    # final projection for the 256-token block this half completed
                    emit_final(2 * qc + half)

    nc.compile()
    return nc


def _get_nc():
    if "nc" not in _CACHE:
        _CACHE["nc"] = _build_nc()
    return _CACHE["nc"]


def _make_in_maps(x, gn_w, gn_b, wq, bq, wk, bk, wv, bv, wo, bo):
    x = np.ascontiguousarray(np.asarray(x, dtype=np.float32)).reshape(B, C, N)
    pairm = np.zeros((P, P), dtype=np.float32)
    idx = np.arange(P)
    pairm[idx[:, None] // 2 == idx[None, :] // 2] = 0.5
    common = {
        "wqt": np.ascontiguousarray(np.asarray(wq, np.float32).T),
        "wkt": np.ascontiguousarray(np.asarray(wk, np.float32).T),
        "wvt": np.ascontiguousarray(np.asarray(wv, np.float32).T),
        "wot": np.ascontiguousarray(np.asarray(wo, np.float32).T),
        "bq": np.asarray(bq, np.float32),
        "bk": np.asarray(bk, np.float32),
        "bv": np.asarray(bv, np.float32),
        "bo": np.asarray(bo, np.float32),
        "gnw": np.asarray(gn_w, np.float32),
        "gnb": np.asarray(gn_b, np.float32),
        "pairm": pairm,
    }
    in_maps = []
    for core in range(NCORES):
        b, half = divmod(core, 2)
        xs = np.roll(x[b], -NL * half, axis=1) if half else x[b]
        in_maps.append({**common, "x": np.ascontiguousarray(xs)})
    return in_maps


def kernel(x, gn_w, gn_b, wq, bq, wk, bk, wv, bv, wo, bo):
    from concourse.bass_utils import run_bass_kernel_spmd

    nc = _get_nc()
    in_maps = _make_in_maps(x, gn_w, gn_b, wq, bq, wk, bk, wv, bv, wo, bo)
    res = run_bass_kernel_spmd(nc, in_maps, core_ids=list(range(NCORES)))
    _CACHE["last_result"] = res

    out = np.empty((B, C, N), dtype=np.float32)
    for core in range(NCORES):
        b, half = divmod(core, 2)
        out[b][:, NL * half:NL * (half + 1)] = res.results[core]["out"]
    return out.reshape(B, C, HH, WW)
